# revision 22
# baseline (speedup 1.0000x reference)
"""AdaAttN Trainium2 kernel — 8-core SPMD, no collectives.

Problem: for each batch image b (4 total):
  F = f_w @ c_1x[b] + f_b; G = g_w @ s_1x[b] + g_b; Hs = h_w @ s_x[b] + h_b
  S = softmax(F^T G, rows)  [4096 x 4096]
  mean = S @ Hs^T; e2 = S @ (Hs*Hs)^T; std = sqrt(relu(e2 - mean^2))
  out[b] = std^T * c_x[b] + mean^T

Sharding: core = 2*b + qh handles batch b, query half qh (2048 queries).

Key design points:
- Host-side weight fusion: F^T G = c_1x^T (f_w^T g_w) s_1x + (per-query
  const, softmax-invariant, dropped) + t[m] where t = (g_w^T f_b)^T s_1x
  is computed on host and rides the exp's per-partition bias. This deletes
  the F projection (64 MMs/core) and all projection bias riders.
- h_b is folded into the epilogue: var is shift-invariant, so PV runs on
  unbiased Hs and mean += h_b happens as a (hidden) DVE add against a
  host-replicated [128,C] tile.
- S^T is computed directly ([m_part, q_free]) so P = exp(S^T + t - 80)
  lands in the transposed layout the PV matmul needs; softmax max-
  subtraction is replaced by a global shift (safe in bf16/f32 range).
- The softmax row-sum rides a ones column spliced into the middle of the
  Hshi value tile: the PV mean matmul runs as two halves (N=257 + N=256)
  so the row-sum costs ~2 extra streaming cycles per m-tile instead of a
  separate FD=1 matmul (~26ns dispatch floor) — saves ~12us/core.
- fp16 S-chain; PV: P bf16 stationary; mean rhs Hshi (bf16), e2 rhs
  fp16(Hshi^2) so the e2 - mean^2 cancellation keeps a ~2^-12 floor.
- Epilogue fused with scalar_tensor_tensor ((e2*rinv) - mean^2 in one DVE
  op); fp8/DoubleRow was evaluated and is numerically dead here (softmax
  is near-one-hot, so e4m3 Hs rounding lands ~2% directly on the output,
  vs the 2e-2 gate; hi-lo fp8 splits cost exactly as much as bf16).
Measured on HW: rel err 5.40e-3; exec ~425-428us (from 462us baseline),
PE-bound: ~402us busy at 2.4GHz vs ~387us instruction floor, plus ~7us
NEFF preamble, ~4us head DMA, ~6us tail epilogue, ~9us teardown.
Occasional runs power-throttle to ~2.0GHz (+25us) — chip state, not
kernel-dependent.
"""

import os
import sys

os.environ.setdefault("MYCRO_LOCAL_CACHE", "1")
if "/opt/trn_rl_repo" not in sys.path:
    sys.path.insert(0, "/opt/trn_rl_repo")

import numpy as np

import concourse.bass as bass  # noqa: F401  (engine types)
import concourse.mybir as mybir
import concourse.tile as tile
from concourse import bacc
from concourse.bass_utils import run_bass_kernel_spmd

FP16 = mybir.dt.float16
BF16 = mybir.dt.bfloat16
F32 = mybir.dt.float32
AF = mybir.ActivationFunctionType

B = 4
C = 512      # value channels
KP = 512     # key/query channels
M = 4096     # keys per image
NQ = 2048    # queries per core
KC = 4       # contraction chunks of 128
MT = 32      # m-tiles of 128
MCH = 4      # 1024-key input chunks for the projections
QW = 512     # query-block width
NBLK = NQ // QW   # 4 query blocks
QTB = QW // 128   # 4 q-tiles per block
QT = NQ // 128    # 16 q-tiles
SHIFT = 80.0
CH = C // 2  # channel half for the split mean matmul

PT_BUFS = 2 * MT + 4


def _build_program(nc):
    d_c1x = nc.dram_tensor("c1x", [128, KC, NQ], FP16, kind="ExternalInput")
    d_s1x = nc.dram_tensor("s1x", [128, KC, M], FP16, kind="ExternalInput")
    d_sx = nc.dram_tensor("sx", [128, KC, M], FP16, kind="ExternalInput")
    d_cxT = nc.dram_tensor("cxT", [QT, 128, C], F32, kind="ExternalInput")
    d_wT = nc.dram_tensor("wT", [128, KC, KP], FP16, kind="ExternalInput")
    d_hwT = nc.dram_tensor("hwT", [128, KC, C], FP16, kind="ExternalInput")
    d_tm = nc.dram_tensor("tm", [128, MT], F32, kind="ExternalInput")
    d_hbt = nc.dram_tensor("hbt", [128, C], F32, kind="ExternalInput")
    d_out = nc.dram_tensor("out", [QT, 128, C], F32, kind="ExternalOutput")

    with tile.TileContext(nc) as tc:
        with (
            tc.tile_pool(name="persist", bufs=1) as persist,
            tc.tile_pool(name="psS", bufs=2, space="PSUM") as psS,
            tc.tile_pool(name="psM", bufs=2, space="PSUM") as psM,
        ):
            Gp = persist.tile([128, KC, M], FP16, tag="Gp")
            # Hshi holds Hs^T with a ones column spliced in at position CH:
            # [Hs[0:256] | 1.0 | Hs[256:512]]. The PV mean matmul then runs
            # as two halves (N=257 + N=256) and the row-sum rides the ones
            # column of the first half for ~2 extra cycles instead of a
            # separate FD=1 matmul per m-tile (~26ns dispatch floor each).
            Hshi = persist.tile([128, MT, C + 1], BF16, tag="Hshi")
            nc.vector.memset(Hshi[:, :, CH : CH + 1], 1.0)
            Hs2f = persist.tile([128, MT, C], FP16, tag="Hs2f")
            c1xp = persist.tile([128, KC, NQ], FP16, tag="c1xp")
            tmT = persist.tile([128, MT], F32, tag="tmT")
            hbt = persist.tile([128, C], F32, tag="hbt")

            # ---------------- projections ----------------
            with (
                tc.tile_pool(name="wpool", bufs=1) as wpool,
                tc.tile_pool(name="stage", bufs=3) as stage,
            ):
                wT = wpool.tile([128, KC, KP], FP16, tag="wT")
                nc.sync.dma_start(wT[:], d_wT[:])
                hwT = wpool.tile([128, KC, C], FP16, tag="hwT")

                MW = M // MCH  # 1024
                # G2 = (f_w^T g_w) @ s_1x  -> Gp [k_part, m]   (bias-free)
                # s1x streams in 512-key chunks so the first MM group can
                # start after ~1MB of DMA.
                s1t = []
                for mc in range(MCH):
                    t = stage.tile([128, KC, MW], FP16, tag="s1x")
                    for h in range(2):
                        nc.sync.dma_start(
                            t[:, :, h * 512 : (h + 1) * 512],
                            d_s1x[:, :, mc * MW + h * 512 : mc * MW + (h + 1) * 512],
                        )
                    s1t.append(t)
                nc.sync.dma_start(hwT[:], d_hwT[:])
                sxt = []
                for mc in range(MCH):
                    t = stage.tile([128, KC, MW], FP16, tag="sx")
                    nc.sync.dma_start(t[:], d_sx[:, :, mc * MW : (mc + 1) * MW])
                    sxt.append(t)
                for q4 in range(NQ // 512):
                    nc.sync.dma_start(
                        c1xp[:, :, q4 * 512 : (q4 + 1) * 512],
                        d_c1x[:, :, q4 * 512 : (q4 + 1) * 512],
                    )
                nc.sync.dma_start(tmT[:], d_tm[:])
                nc.sync.dma_start(hbt[:], d_hbt[:])

                for mc in range(MCH):
                    for mb in range(MW // 512):
                        for kt in range(KC):
                            ps = psS.tile([128, 512], F32, tag="s")
                            for ci in range(KC):
                                nc.tensor.matmul(
                                    ps[:],
                                    wT[:, ci, kt * 128 : (kt + 1) * 128],
                                    s1t[mc][:, ci, mb * 512 : (mb + 1) * 512],
                                    start=(ci == 0),
                                    stop=(ci == KC - 1),
                                )
                            ms = mc * MW + mb * 512
                            nc.scalar.copy(Gp[:, kt, ms : ms + 512], ps[:])

                # HsT = (h_w @ s_x)^T  -> [m_part, c] bf16 + fp16 square
                for mc in range(MCH):
                    for mt in range(MW // 128):
                        mg = mc * (MW // 128) + mt
                        ps = psS.tile([128, 512], F32, tag="s")
                        for ci in range(KC):
                            nc.tensor.matmul(
                                ps[:],
                                sxt[mc][:, ci, mt * 128 : (mt + 1) * 128],
                                hwT[:, ci, :],
                                start=(ci == 0),
                                stop=(ci == KC - 1),
                            )
                        nc.scalar.copy(Hshi[:, mg, 0:CH], ps[:, 0:CH])
                        nc.scalar.copy(Hshi[:, mg, CH + 1 : C + 1], ps[:, CH:C])
                        nc.vector.tensor_mul(
                            Hs2f[:, mg, 0:CH],
                            Hshi[:, mg, 0:CH], Hshi[:, mg, 0:CH],
                        )
                        nc.vector.tensor_mul(
                            Hs2f[:, mg, CH:C],
                            Hshi[:, mg, CH + 1 : C + 1],
                            Hshi[:, mg, CH + 1 : C + 1],
                        )

            # ---------------- attention ----------------
            with (
                tc.tile_pool(name="pt", bufs=PT_BUFS) as ptp,
                tc.tile_pool(name="cxp", bufs=3) as cxp,
                tc.tile_pool(name="aepi", bufs=3) as aepi,
            ):
                def s_block(qb):
                    qs = qb * QW
                    pts = []
                    for mt in range(MT):
                        ps = psS.tile([128, QW], F32, tag="s")
                        for kc in range(KC):
                            nc.tensor.matmul(
                                ps[:],
                                Gp[:, kc, mt * 128 : (mt + 1) * 128],
                                c1xp[:, kc, qs : qs + QW],
                                start=(kc == 0),
                                stop=(kc == KC - 1),
                            )
                        pt = ptp.tile([128, QW], BF16, tag="pt")
                        nc.scalar.activation(
                            pt[:], ps[:], AF.Exp, bias=tmT[:, mt : mt + 1]
                        )
                        pts.append(pt)
                    return pts

                # prefetch c_x one q-tile ahead
                cx_tiles = {}
                for g in range(2):
                    cx_tiles[g] = cxp.tile([128, C], F32, tag="cx", name="cxt")
                    nc.sync.dma_start(cx_tiles[g][:], d_cxT[g])

                # software-pipelined: emit S^T of block qb+1 before PV of qb
                pts_by_block = {0: s_block(0)}
                for qb in range(NBLK):
                    if qb + 1 < NBLK:
                        pts_by_block[qb + 1] = s_block(qb + 1)
                    pts = pts_by_block.pop(qb)
                    for qt in range(QTB):
                        g = qb * QTB + qt
                        pma = psM.tile([128, CH + 1], F32, tag="ma", name="pma")
                        pmb = psM.tile([128, CH], F32, tag="mb", name="pmb")
                        pmc = psM.tile([128, C], F32, tag="mc", name="pmc")
                        for mt in range(MT):
                            lhs = pts[mt][:, qt * 128 : (qt + 1) * 128]
                            first = mt == 0
                            last = mt == MT - 1
                            nc.tensor.matmul(
                                pma[:], lhs, Hshi[:, mt, 0 : CH + 1],
                                start=first, stop=last,
                            )
                            nc.tensor.matmul(
                                pmb[:], lhs, Hshi[:, mt, CH + 1 : C + 1],
                                start=first, stop=last,
                            )
                            nc.tensor.matmul(
                                pmc[:], lhs, Hs2f[:, mt, :],
                                start=first, stop=last,
                            )

                        if g + 2 < QT:
                            cx_tiles[g + 2] = cxp.tile(
                                [128, C], F32, tag="cx", name="cxt"
                            )
                            nc.sync.dma_start(cx_tiles[g + 2][:], d_cxT[g + 2])
                        cxt = cx_tiles.pop(g)
                        rinv = aepi.tile([128, 1], F32, tag="rinv")
                        nc.vector.reciprocal(rinv[:], pma[:, CH : CH + 1])
                        for h in range(2):
                            hs = slice(h * CH, (h + 1) * CH)
                            pmean = pma[:, 0:CH] if h == 0 else pmb[:]
                            mean = aepi.tile(
                                [128, CH], F32, tag="mean", name="mean"
                            )
                            nc.vector.tensor_scalar_mul(mean[:], pmean, rinv[:])
                            t1 = aepi.tile([128, CH], F32, tag="t1", name="t1")
                            nc.vector.tensor_mul(t1[:], mean[:], mean[:])
                            # t1 = e2 - mean^2 = (pmc * rinv) - mean^2, fused
                            nc.vector.scalar_tensor_tensor(
                                t1[:], pmc[:, hs], rinv[:], t1[:],
                                mybir.AluOpType.mult, mybir.AluOpType.subtract,
                            )
                            nc.vector.tensor_scalar_max(t1[:], t1[:], 0.0)
                            nc.scalar.sqrt(t1[:], t1[:])
                            # mean += h_b (independent of the std chain)
                            nc.vector.tensor_add(mean[:], mean[:], hbt[:, hs])
                            ot = aepi.tile([128, CH], F32, tag="ot", name="ot")
                            nc.vector.tensor_mul(ot[:], t1[:], cxt[:, hs])
                            nc.vector.tensor_add(ot[:], ot[:], mean[:])
                            nc.sync.dma_start(d_out[g, :, hs], ot[:])
    return nc


_NC = None


def build():
    global _NC
    if _NC is None:
        nc = bacc.Bacc(
            "TRN2", target_bir_lowering=False, debug=False, enable_asserts=True
        )
        _build_program(nc)
        nc.compile()
        _NC = nc
    return _NC


def make_in_maps(inputs):
    c_x = np.asarray(inputs["c_x"], np.float32).reshape(B, C, M)
    s_x = np.asarray(inputs["s_x"], np.float32).reshape(B, C, M)
    c_1x = np.asarray(inputs["c_1x"], np.float32).reshape(B, KP, M)
    s_1x = np.asarray(inputs["s_1x"], np.float32).reshape(B, KP, M)
    f_w = np.asarray(inputs["f_w"], np.float64)
    g_w = np.asarray(inputs["g_w"], np.float64)
    h_w = np.asarray(inputs["h_w"], np.float32)
    f_b = np.asarray(inputs["f_b"], np.float64)
    g_b = np.asarray(inputs["g_b"], np.float64)  # noqa: F841 (softmax-invariant)
    h_b = np.asarray(inputs["h_b"], np.float32)

    def chunked(x):
        # [512, n] -> [128, 4, n]
        return np.ascontiguousarray(x.reshape(KC, 128, -1).transpose(1, 0, 2))

    # W = f_w^T g_w fused on host; stationary layout needs W^T = g_w^T f_w
    wT = chunked((g_w.T @ f_w).astype(np.float16))
    hwT = chunked(h_w.T.astype(np.float16))
    u = (g_w.T @ f_b).astype(np.float32)        # t[m] = u . s_1x[b][:, m]
    hbt = np.ascontiguousarray(
        np.broadcast_to(h_b.reshape(1, C), (128, C)).astype(np.float32)
    )

    in_maps = []
    for core in range(8):
        b, qh = divmod(core, 2)
        qs = slice(qh * NQ, (qh + 1) * NQ)
        t = (u @ s_1x[b]).astype(np.float32) - SHIFT      # [M]
        tm = np.ascontiguousarray(t.reshape(MT, 128).T)   # [128, MT]
        in_maps.append(
            {
                "c1x": chunked(c_1x[b][:, qs].astype(np.float16)),
                "s1x": chunked(s_1x[b].astype(np.float16)),
                "sx": chunked(s_x[b].astype(np.float16)),
                "cxT": np.ascontiguousarray(c_x[b][:, qs].T).reshape(QT, 128, C),
                "wT": wT,
                "hwT": hwT,
                "tm": tm,
                "hbt": hbt,
            }
        )
    return in_maps


def assemble_out(results):
    outs = []
    for b in range(B):
        lo = results[2 * b]["out"].reshape(NQ, C)
        hi = results[2 * b + 1]["out"].reshape(NQ, C)
        full = np.concatenate([lo, hi], axis=0)  # [4096, 512] (q, c)
        outs.append(full.T.reshape(C, 64, 64))
    return np.stack(outs).astype(np.float32)


def _install_ntff_hook():
    """Register the axon NTFF profiling hook (absent from this image's antenv)
    so run_bass_kernel_spmd(trace=True) can return exec_time_ns."""
    try:
        from antenv.axon_hooks import get_axon_ntff_profile_hook  # noqa: F401

        return True
    except ImportError:
        pass
    import contextlib
    import ctypes
    import types

    so_path = "/opt/axon/libaxon_pjrt.so"
    if not os.path.exists(so_path):
        return False
    lib = ctypes.CDLL(so_path)
    if not hasattr(lib, "axon_start_nrt_profile"):
        return False
    lib.axon_start_nrt_profile.argtypes = [
        ctypes.POINTER(ctypes.c_int64),
        ctypes.c_size_t,
    ]
    lib.axon_start_nrt_profile.restype = ctypes.c_int64
    lib.axon_stop_nrt_profile.argtypes = [ctypes.c_char_p]
    lib.axon_stop_nrt_profile.restype = ctypes.c_int64

    @contextlib.contextmanager
    def _hook(output_dir, device_ids):
        import jax

        jax.devices()
        if device_ids:
            ids = (ctypes.c_int64 * len(device_ids))(*device_ids)
            rc = lib.axon_start_nrt_profile(ids, len(device_ids))
        else:
            rc = lib.axon_start_nrt_profile(None, 0)
        if rc != 0:
            raise RuntimeError(f"axon_start_nrt_profile rc={rc}")
        try:
            yield
        finally:
            n = lib.axon_stop_nrt_profile(str(output_dir).encode())
            print(f"profile: {n} file(s) written to {output_dir}", file=sys.stderr)

    holder = {"hook": _hook}
    mod = types.ModuleType("antenv.axon_hooks")
    mod.set_axon_ntff_profile_hook = lambda h: holder.__setitem__("hook", h)
    mod.get_axon_ntff_profile_hook = lambda: holder["hook"]
    sys.modules["antenv.axon_hooks"] = mod
    import antenv

    antenv.axon_hooks = mod
    return True


def run(inputs, trace=False, **kwargs):
    nc = build()
    in_maps = make_in_maps(inputs)
    if trace:
        _install_ntff_hook()
    res = run_bass_kernel_spmd(
        nc, in_maps, core_ids=list(range(8)), trace=trace, **kwargs
    )
    return assemble_out(res.results), res.exec_time_ns


def kernel(**inputs):
    out, _ = run(inputs)
    return out
